# revision 47
# baseline (speedup 1.0000x reference)
"""M2 convection (SE(2) trilinear warp) Trainium2 kernel.

out[b,c,k,i,j] = x[b,c] trilinearly sampled at (theta_k, i, j) . g0[c]^{-1}.

Structure exploited: for fixed (c,k) the warp is a uniform translation —
theta taps are two whole slices (a_k, a_k+1) with constant weights, the y
taps are a per-row integer shift + 2-tap blend (exactly encoded in a banded
matrix applied on the PE, theta weight folded in), and the x taps are a
free-dim shift + 2-tap blend. Runtime-register APs are unavailable on this
execution path, so the x 2-tap blend is computed at every candidate shift
(fixed taps j, j+1 over a zero-padded PSUM tile) and the host selects each
(c,k)'s shifted window from a slightly padded output.

Weight folding: the y matrices carry wt0 (theta tap-0 weight) and
c0 = 1-fmid (x tap-0 weight), so the theta and x blends are each a single
scalar_tensor_tensor with ratio scalars ft/wt0 and fmid/c0. Matmuls run in
float32r (full-rate PE mode; ~1e-3 relative precision, far inside the 2e-2
gate).

Sharding: channels across 8 cores (2 channels/core, no communication).
The kernel's slice-slot m corresponds to output k with a_k == m; the host
unpermutes along theta at the end.
"""
import os
import sys
import numpy as np

sys.path.insert(0, "/opt/trn_rl_repo")

import concourse.mybir as mybir  # noqa: E402
from concourse import bacc, bass_utils  # noqa: E402
from concourse.tile import TileContext  # noqa: E402

TWO_PI = 2.0 * np.pi
B, C, Or, H, W = 4, 16, 8, 256, 256
N_CORES = 8
C_LOC = C // N_CORES          # channels per core
N_CK = C_LOC * Or             # (c_local, m) pairs per core


def _reference_tables(g0):
    """Replicate the reference's f32 index/weight math (jax on CPU so the
    rounding matches the jax reference bit-for-bit)."""
    import jax
    import jax.numpy as jnp

    with jax.default_device(jax.devices("cpu")[0]):
        g0 = jnp.asarray(g0, dtype=jnp.float32)
        x0, y0, th0 = g0[:, 0], g0[:, 1], g0[:, 2]
        k = jnp.arange(Or, dtype=jnp.float32)
        alpha = k[None, :] * (TWO_PI / Or) - th0[:, None]
        ca, sa = jnp.cos(alpha), jnp.sin(alpha)
        dx = ca * x0[:, None] - sa * y0[:, None]
        dy = sa * x0[:, None] + ca * y0[:, None]
        t = k[None, :] - th0[:, None] * (Or / TWO_PI)
        xs = jnp.arange(W, dtype=jnp.float32)[None, None, :] - dx[:, :, None]
        ys = jnp.arange(H, dtype=jnp.float32)[None, None, :] - dy[:, :, None]
        tf = jnp.floor(t)
        ft = t - tf
        t0i = tf.astype(jnp.int32)
        xf = jnp.floor(xs)
        fx = xs - xf
        x0i = xf.astype(jnp.int32)
        yf = jnp.floor(ys)
        fy = ys - yf
        y0i = yf.astype(jnp.int32)
        return dict(
            ft=np.asarray(ft), t0i=np.asarray(t0i),
            fx=np.asarray(fx), x0i=np.asarray(x0i),
            fy=np.asarray(fy), y0i=np.asarray(y0i),
        )


def _x_shift(tabs, c, k):
    return int(tabs["x0i"][c, k][W // 2]) - W // 2


def _pads(tabs):
    hs = [_x_shift(tabs, c, k) for c in range(C) for k in range(Or)]
    padl = -min(hs) + 2
    padr = max(hs) + 1 + 2
    return max(padl, 2), max(padr, 2)


def _core_tables(tabs, channels, padl, nu):
    """Build per-core kernel input tensors from the reference tables.

    Returns (mats, rscal, wxr, hvals, slot_to_k): slot_to_k[c_local][m] is
    the output-theta index computed in slice-slot m; hvals its x shift.
    """
    mats = np.zeros((128, C_LOC, Or, 2, 2, 128), dtype=np.float32)
    rscal = np.zeros((128, N_CK), dtype=np.float32)
    wxr = np.zeros((128, N_CK), dtype=np.float32)
    hvals = np.zeros((C_LOC, Or), dtype=np.int64)
    slot_to_k = np.zeros((C_LOC, Or), dtype=np.int64)

    for cl, c in enumerate(channels):
        a = np.mod(tabs["t0i"][c], Or)          # [Or] A-slice per out-k
        assert sorted(a.tolist()) == list(range(Or)), f"theta map not a bijection: {a}"
        k_of_m = np.zeros(Or, dtype=np.int64)
        k_of_m[a] = np.arange(Or)
        slot_to_k[cl] = k_of_m
        for m in range(Or):
            k = int(k_of_m[m])
            cki = cl * Or + m
            ft = np.float32(tabs["ft"][c, k])
            wt0 = np.float32(1.0) - ft
            # blend: t = slot_m + r * slot_{m+1}; (1-ft) folded into mats
            rscal[:, cki] = np.float32(ft / wt0) if wt0 > 0 else np.float32(0)
            # --- x scalars (c0 = 1-fmid folded into mats) ---
            x0i = tabs["x0i"][c, k]             # [W] int
            fx = tabs["fx"][c, k]               # [W] f32
            h = _x_shift(tabs, c, k)
            nonuni = np.abs(x0i - (np.arange(W) + h)).max()
            assert nonuni <= 1, f"x shift non-uniformity {nonuni} too large"
            fmid = np.float32(0.5) * (fx.min() + fx.max())
            c0 = np.float32(1.0) - fmid
            wxr[:, cki] = np.float32(fmid / c0)
            assert 0 <= padl + h and padl + h + 1 + W <= nu, f"x shift {h} vs pads"
            hvals[cl, m] = h
            # --- y matrices (per-row exact; wt0 and c0 folded in) ---
            y0i = tabs["y0i"][c, k]             # [H] int
            fy = tabs["fy"][c, k]               # [H] f32
            for dyc in (0, 1):
                wrow = (fy if dyc else (np.float32(1.0) - fy)).astype(np.float32)
                wrow = (wrow * wt0 * c0).astype(np.float32)
                r = y0i + dyc                    # src row per out row i
                valid = (r >= 0) & (r < H)
                i_idx = np.nonzero(valid)[0]
                rv = r[i_idx]
                mats[rv % 128, cl, m, i_idx // 128, rv // 128, i_idx % 128] += \
                    wrow[i_idx]
    return mats, rscal, wxr, hvals, slot_to_k


def _build_program(padl, padr):
    nu = W + padl + padr        # padded PSUM width
    nv = nu - 1                 # output candidate width
    nc = bacc.Bacc("TRN2", num_devices=N_CORES)
    f32 = mybir.dt.float32
    f16 = mybir.dt.float16
    x_d = nc.dram_tensor("xs", [B, C_LOC, Or, H, W], f16, kind="ExternalInput")
    m_d = nc.dram_tensor("mats", [128, C_LOC, Or, 2, 2, 128], f16, kind="ExternalInput")
    r_d = nc.dram_tensor("rscal", [128, N_CK], f32, kind="ExternalInput")
    w_d = nc.dram_tensor("wx", [128, N_CK], f32, kind="ExternalInput")
    o_d = nc.dram_tensor("o", [B, C_LOC, Or, H, nv], f16, kind="ExternalOutput")

    # Engine split (real-ISA constraints: Pool has no STT and no PSUM
    # access; ACT is 1x-rate single-tensor only).  Per m:
    #   z1 = x[m+1]*r      DVE ts (194ns; 2-of-8 on ACT to balance)
    #   t  = x[m] + z1     DVE tt (327)
    #   4 matmuls          PE   (476)
    #   V  = copy(U)       ACT  (657, PSUM f32 -> SBUF f16)
    #   z2 = V[1:]*s       Pool ts (881; 1-of-8 on ACT)
    #   out = V[:-1] + z2  DVE tt (341)
    # Busy/core: DVE ~52us, Pool ~49us, ACT ~51us, PE ~31us, DMA ~55us.
    # DMA granularity: each DMACopy costs ~650ns of serial SP dispatch and
    # gates the next transfer, so transfers are kept coarse (>=0.5MB).
    Z1_ACT = set(_CFG.get("z1_act", {3}))   # z1 on ACT for these m
    Z2_ACT = set(_CFG.get("z2_act", {1, 5}))  # z2 on ACT for these m
    # Last iteration: spread z2 wide so the Pool z2 chain doesn't become a
    # serial ~6us drain after the final copies.
    Z2_LAST = _CFG.get(
        "z2_last", {1: "act", 4: "act", 7: "act", 2: "dve", 6: "dve"})
    AL = mybir.AluOpType

    with TileContext(nc) as tc:
        with tc.tile_pool(name="const", bufs=1) as cpool, \
             tc.tile_pool(name="xin", bufs=2) as xpool, \
             tc.tile_pool(name="work", bufs=8) as wpool, \
             tc.tile_pool(name="oout", bufs=2) as opool, \
             tc.tile_pool(name="psum", bufs=1, space="PSUM") as psum:
            rt = cpool.tile([128, N_CK], f32)
            wt = cpool.tile([128, N_CK], f32)
            nc.sync.dma_start(out=rt[:], in_=r_d.ap())
            nc.sync.dma_start(out=wt[:], in_=w_d.ap())
            # Per-(cl,m) matrix tiles: allocated here, loaded later (after
            # iteration 0's x slices) so the first theta blends aren't stuck
            # behind the whole 2.1MB constant load on the serial DMA bus.
            mt = {}
            for cl in range(C_LOC):
                for m in range(Or):
                    mt[cl, m] = cpool.tile([128, 2, 2, 128], f16,
                                           name=f"mt{cl}_{m}")

            def load_mats(cl, half):
                lo = half * (Or // 2)
                for m in range(lo, lo + Or // 2):
                    nc.sync.dma_start(out=mt[cl, m][:], in_=m_d.ap()[:, cl, m])

            # 8 persistent PSUM tiles, one bank each ([128,2,W] f32 = 2KB per
            # partition-row pair): the matmuls write the exact W window and
            # PE can run up to 8 groups ahead of the copies.
            U_tiles = []
            for i in range(8):
                U = psum.tile([128, 2, W], f32, tag=f"U{i}", name=f"U{i}")
                U_tiles.append(U)
            uidx = 0
            # Persistent V ring: pads zeroed once (Pool memsets); the ACT
            # copies then only write the W-wide middle each reuse.
            V_tiles = []
            for i in range(16):
                V = cpool.tile([128, 2, nu], f16, name=f"V{i}")
                for u in range(2):
                    nc.gpsimd.memset(V[:, u, 0:padl], 0.0)
                    nc.gpsimd.memset(V[:, u, padl + W:nu], 0.0)
                V_tiles.append(V)
            vidx = 0

            iters = [(b, cl) for b in range(B) for cl in range(C_LOC)]

            def load_x(b, cl):
                x_sb = xpool.tile([128, Or, 2, W], f16, tag="x_sb",
                                  name="x_sb")
                src = x_d.ap()[b, cl].rearrange("k (u p) j -> p k u j", p=128)
                nc.sync.dma_start(out=x_sb[:], in_=src)
                return x_sb

            def emit_s1(cl, xs):
                """Theta blends for one (b,cl); returns t tiles."""
                ts = []
                for m in range(Or):
                    cki = cl * Or + m
                    z = wpool.tile([128, 2, W], f16, tag="z", name="z",
                                   bufs=4)
                    if m in Z1_ACT:
                        nc.scalar.mul(z[:], xs[:, (m + 1) % Or],
                                      rt[:, cki:cki + 1])
                    else:
                        nc.vector.tensor_scalar_mul(
                            z[:], xs[:, (m + 1) % Or], rt[:, cki:cki + 1])
                    # Two iterations' worth of t tiles live at once (the
                    # software pipeline emits S1(i+1) before tail(i)).
                    t = wpool.tile([128, 2, W], f16, tag="t", name="t",
                                   bufs=16)
                    nc.vector.tensor_tensor(
                        out=t[:], in0=xs[:, m], in1=z[:], op=AL.add)
                    ts.append(t)
                return ts

            def emit_tail(b, cl, ts, last=False):
                nonlocal uidx, vidx
                # Two half-size output tiles so the second store DMA can
                # start while m=4..7 are still being computed.
                out_a = opool.tile([128, Or // 2, 2, nv], f16, tag="out_a",
                                   name="out_a")
                out_b = opool.tile([128, Or // 2, 2, nv], f16, tag="out_b",
                                   name="out_b")
                for m in range(Or):
                    cki = cl * Or + m
                    U = U_tiles[uidx % 8]
                    uidx += 1
                    for u in range(2):
                        for v in range(2):
                            nc.tensor.matmul(
                                U[:, u],
                                mt[cl, m][:, u, v],
                                ts[m][:, v],
                                start=(v == 0), stop=(v == 1))
                    # ACT stages the W-wide row PSUM->SBUF (f32->f16); the
                    # 16-deep ring decouples iteration i+1's copies from
                    # iteration i's x-blend consumers.
                    V = V_tiles[vidx % 16]
                    vidx += 1
                    nc.scalar.copy(V[:, :, padl:padl + W], U[:])
                    z2 = wpool.tile([128, 2, nv], f16, tag="z2", name="z2",
                                    bufs=16)
                    eng = "pool"
                    if last:
                        eng = Z2_LAST.get(m, "pool")
                    elif m in Z2_ACT:
                        eng = "act"
                    if eng == "act":
                        nc.scalar.mul(z2[:], V[:, :, 1:nu], wt[:, cki:cki + 1])
                    elif eng == "dve":
                        nc.vector.tensor_scalar_mul(
                            z2[:], V[:, :, 1:nu], wt[:, cki:cki + 1])
                    else:
                        nc.gpsimd.tensor_scalar_mul(
                            z2[:], V[:, :, 1:nu], wt[:, cki:cki + 1])
                    out_sb = out_a if m < Or // 2 else out_b
                    nc.vector.tensor_tensor(
                        out=out_sb[:, m % (Or // 2)], in0=V[:, :, 0:nv],
                        in1=z2[:], op=AL.add)
                    q = 2 if (last and _CFG.get("out4_last", True)) else 4
                    if (m + 1) % q == 0:
                        lo = m + 1 - q
                        dst = o_d.ap()[b, cl, lo:lo + q].rearrange(
                            "k (u p) j -> p k u j", p=128)
                        nc.sync.dma_start(
                            out=dst,
                            in_=out_sb[:, lo % (Or // 2):lo % (Or // 2) + q])

            # Software pipeline: emit iteration i+1's loads + theta blends
            # before iteration i's matmul/copy/x-blend block, so each
            # engine's in-order queue always has ready work at block edges.
            # Iteration 0 is fine-grained: half the x slices, then the first
            # mats chunk, then the rest — so the pipe fills ~2us earlier.
            if _CFG.get("fine_start", True):
                xs = xpool.tile([128, Or, 2, W], f16, tag="x_sb", name="x_sb")
                b0, c0 = iters[0]
                for half in range(2):
                    lo = half * (Or // 2)
                    src = x_d.ap()[b0, c0, lo:lo + Or // 2].rearrange(
                        "k (u p) j -> p k u j", p=128)
                    nc.sync.dma_start(out=xs[:, lo:lo + Or // 2], in_=src)
                    load_mats(0, half)
                ts = emit_s1(iters[0][1], xs)
            else:
                xs = load_x(*iters[0])
                load_mats(0, 0)
                load_mats(0, 1)
                ts = emit_s1(iters[0][1], xs)
            interleave = _CFG.get("interleave", False)
            for i in range(len(iters)):
                last = i == len(iters) - 1
                if i + 1 < len(iters):
                    xs_n = load_x(*iters[i + 1])
                    if i == 0:
                        load_mats(1, 0)
                        load_mats(1, 1)
                    if interleave:
                        ts_n = emit_s1_interleaved(
                            iters[i + 1][1], xs_n,
                            iters[i][0], iters[i][1], ts, last)
                    else:
                        ts_n = emit_s1(iters[i + 1][1], xs_n)
                        emit_tail(iters[i][0], iters[i][1], ts, last=last)
                else:
                    ts_n = None
                    emit_tail(iters[i][0], iters[i][1], ts, last=last)
                ts = ts_n
    nc.compile()
    return nc


_CFG = {}                         # build-config knobs (sim search)
_NC_CACHE = {}


def kernel(x, g0):
    x = np.ascontiguousarray(np.asarray(x, dtype=np.float32))
    g0 = np.asarray(g0, dtype=np.float32)
    tabs = _reference_tables(g0)
    padl, padr = _pads(tabs)
    nu = W + padl + padr

    if (padl, padr) not in _NC_CACHE:
        _NC_CACHE[(padl, padr)] = _build_program(padl, padr)
    nc = _NC_CACHE[(padl, padr)]

    in_maps = []
    slot_maps = []
    for core in range(N_CORES):
        channels = list(range(core * C_LOC, (core + 1) * C_LOC))
        mats, rscal, wxr, hvals, slot_to_k = _core_tables(tabs, channels, padl, nu)
        in_maps.append({
            "xs": np.ascontiguousarray(
                x[:, channels[0]:channels[-1] + 1]).astype(np.float16),
            "mats": mats.astype(np.float16),
            "rscal": rscal, "wx": wxr,
        })
        assert float(rscal.max()) < 3e4 and float(wxr.max()) < 3e4
        slot_maps.append((slot_to_k, hvals))

    res = bass_utils.run_bass_kernel_spmd(
        nc, in_maps, core_ids=list(range(N_CORES)),
        trace=bool(int(os.environ.get("KERNEL_TRACE", "0"))))
    kernel.last_results = res

    out = np.empty((B, C, Or, H, W), dtype=np.float32)
    for core in range(N_CORES):
        # [B, C_LOC, Or, H, nv] f16, slot m
        raw = res.results[core]["o"].astype(np.float32)
        slot_to_k, hvals = slot_maps[core]
        for cl in range(C_LOC):
            c = core * C_LOC + cl
            for m in range(Or):
                k = int(slot_to_k[cl, m])
                s = padl + int(hvals[cl, m])
                out[:, c, k] = raw[:, cl, m, :, s:s + W]
    return out



# revision 54
# speedup vs baseline: 1.0348x; 1.0348x over previous
"""M2 convection (SE(2) trilinear warp) Trainium2 kernel.

out[b,c,k,i,j] = x[b,c] trilinearly sampled at (theta_k, i, j) . g0[c]^{-1}.

Structure exploited: for fixed (c,k) the warp is a uniform translation —
theta taps are two whole slices (a_k, a_k+1) with constant weights, the y
taps are a per-row integer shift + 2-tap blend (exactly encoded in a banded
matrix applied on the PE, theta weight folded in), and the x taps are a
free-dim shift + 2-tap blend. Runtime-register APs are unavailable on this
execution path, so the x 2-tap blend is computed at every candidate shift
(fixed taps j, j+1 over a zero-padded PSUM tile) and the host selects each
(c,k)'s shifted window from a slightly padded output.

Weight folding: the y matrices carry wt0 (theta tap-0 weight) and
c0 = 1-fmid (x tap-0 weight), so the theta and x blends are each a single
scalar_tensor_tensor with ratio scalars ft/wt0 and fmid/c0. Matmuls run in
float32r (full-rate PE mode; ~1e-3 relative precision, far inside the 2e-2
gate).

Sharding: channels across 8 cores (2 channels/core, no communication).
The kernel's slice-slot m corresponds to output k with a_k == m; the host
unpermutes along theta at the end.
"""
import os
import sys
import numpy as np

sys.path.insert(0, "/opt/trn_rl_repo")

import concourse.mybir as mybir  # noqa: E402
from concourse import bacc, bass_utils  # noqa: E402
from concourse.tile import TileContext  # noqa: E402

TWO_PI = 2.0 * np.pi
B, C, Or, H, W = 4, 16, 8, 256, 256
N_CORES = 8
C_LOC = C // N_CORES          # channels per core
N_CK = C_LOC * Or             # (c_local, m) pairs per core


def _reference_tables(g0):
    """Replicate the reference's f32 index/weight math (jax on CPU so the
    rounding matches the jax reference bit-for-bit)."""
    import jax
    import jax.numpy as jnp

    with jax.default_device(jax.devices("cpu")[0]):
        g0 = jnp.asarray(g0, dtype=jnp.float32)
        x0, y0, th0 = g0[:, 0], g0[:, 1], g0[:, 2]
        k = jnp.arange(Or, dtype=jnp.float32)
        alpha = k[None, :] * (TWO_PI / Or) - th0[:, None]
        ca, sa = jnp.cos(alpha), jnp.sin(alpha)
        dx = ca * x0[:, None] - sa * y0[:, None]
        dy = sa * x0[:, None] + ca * y0[:, None]
        t = k[None, :] - th0[:, None] * (Or / TWO_PI)
        xs = jnp.arange(W, dtype=jnp.float32)[None, None, :] - dx[:, :, None]
        ys = jnp.arange(H, dtype=jnp.float32)[None, None, :] - dy[:, :, None]
        tf = jnp.floor(t)
        ft = t - tf
        t0i = tf.astype(jnp.int32)
        xf = jnp.floor(xs)
        fx = xs - xf
        x0i = xf.astype(jnp.int32)
        yf = jnp.floor(ys)
        fy = ys - yf
        y0i = yf.astype(jnp.int32)
        return dict(
            ft=np.asarray(ft), t0i=np.asarray(t0i),
            fx=np.asarray(fx), x0i=np.asarray(x0i),
            fy=np.asarray(fy), y0i=np.asarray(y0i),
        )


def _x_shift(tabs, c, k):
    return int(tabs["x0i"][c, k][W // 2]) - W // 2


def _pads(tabs):
    hs = [_x_shift(tabs, c, k) for c in range(C) for k in range(Or)]
    padl = -min(hs) + 2
    padr = max(hs) + 1 + 2
    return max(padl, 2), max(padr, 2)


def _core_tables(tabs, channels, padl, nu):
    """Build per-core kernel input tensors from the reference tables.

    Returns (mats, rscal, wxr, hvals, slot_to_k): slot_to_k[c_local][m] is
    the output-theta index computed in slice-slot m; hvals its x shift.
    """
    mats = np.zeros((128, C_LOC, Or, 2, 2, 128), dtype=np.float32)
    rscal = np.zeros((128, N_CK), dtype=np.float32)
    wxr = np.zeros((128, N_CK), dtype=np.float32)
    hvals = np.zeros((C_LOC, Or), dtype=np.int64)
    slot_to_k = np.zeros((C_LOC, Or), dtype=np.int64)

    for cl, c in enumerate(channels):
        a = np.mod(tabs["t0i"][c], Or)          # [Or] A-slice per out-k
        assert sorted(a.tolist()) == list(range(Or)), f"theta map not a bijection: {a}"
        k_of_m = np.zeros(Or, dtype=np.int64)
        k_of_m[a] = np.arange(Or)
        slot_to_k[cl] = k_of_m
        for m in range(Or):
            k = int(k_of_m[m])
            cki = cl * Or + m
            ft = np.float32(tabs["ft"][c, k])
            wt0 = np.float32(1.0) - ft
            # blend: t = slot_m + r * slot_{m+1}; (1-ft) folded into mats
            rscal[:, cki] = np.float32(ft / wt0) if wt0 > 0 else np.float32(0)
            # --- x scalars (c0 = 1-fmid folded into mats) ---
            x0i = tabs["x0i"][c, k]             # [W] int
            fx = tabs["fx"][c, k]               # [W] f32
            h = _x_shift(tabs, c, k)
            nonuni = np.abs(x0i - (np.arange(W) + h)).max()
            assert nonuni <= 1, f"x shift non-uniformity {nonuni} too large"
            fmid = np.float32(0.5) * (fx.min() + fx.max())
            c0 = np.float32(1.0) - fmid
            wxr[:, cki] = np.float32(fmid / c0)
            assert 0 <= padl + h and padl + h + 1 + W <= nu, f"x shift {h} vs pads"
            hvals[cl, m] = h
            # --- y matrices (per-row exact; wt0 and c0 folded in) ---
            y0i = tabs["y0i"][c, k]             # [H] int
            fy = tabs["fy"][c, k]               # [H] f32
            for dyc in (0, 1):
                wrow = (fy if dyc else (np.float32(1.0) - fy)).astype(np.float32)
                wrow = (wrow * wt0 * c0).astype(np.float32)
                r = y0i + dyc                    # src row per out row i
                valid = (r >= 0) & (r < H)
                i_idx = np.nonzero(valid)[0]
                rv = r[i_idx]
                mats[rv % 128, cl, m, i_idx // 128, rv // 128, i_idx % 128] += \
                    wrow[i_idx]
    return mats, rscal, wxr, hvals, slot_to_k


def _build_program(padl, padr):
    nu = W + padl + padr        # padded PSUM width
    nv = nu - 1                 # output candidate width
    nc = bacc.Bacc("TRN2", num_devices=N_CORES)
    f32 = mybir.dt.float32
    f16 = mybir.dt.float16
    x_d = nc.dram_tensor("xs", [B, C_LOC, Or, H, W], f16, kind="ExternalInput")
    m_d = nc.dram_tensor("mats", [128, C_LOC, Or, 2, 2, 128], f16, kind="ExternalInput")
    r_d = nc.dram_tensor("rscal", [128, N_CK], f32, kind="ExternalInput")
    w_d = nc.dram_tensor("wx", [128, N_CK], f32, kind="ExternalInput")
    o_d = nc.dram_tensor("o", [B, C_LOC, Or, H, nv], f16, kind="ExternalOutput")

    # Engine split (real-ISA constraints: Pool has no STT and no PSUM
    # access; ACT is 1x-rate single-tensor only).  Per m:
    #   z1 = x[m+1]*r      DVE ts (194ns; 2-of-8 on ACT to balance)
    #   t  = x[m] + z1     DVE tt (327)
    #   4 matmuls          PE   (476)
    #   V  = copy(U)       ACT  (657, PSUM f32 -> SBUF f16)
    #   z2 = V[1:]*s       Pool ts (881; 1-of-8 on ACT)
    #   out = V[:-1] + z2  DVE tt (341)
    # Busy/core: DVE ~52us, Pool ~49us, ACT ~51us, PE ~31us, DMA ~55us.
    # DMA granularity: each DMACopy costs ~650ns of serial SP dispatch and
    # gates the next transfer, so transfers are kept coarse (>=0.5MB).
    Z1_ACT = set(_CFG.get("z1_act", {3}))   # z1 on ACT for these m
    Z1_POOL = set(_CFG.get("z1_pool", set()))  # z1 on Pool for these m
    Z2_ACT = set(_CFG.get("z2_act", {1, 5}))  # z2 on ACT for these m
    # Last iteration: spread z2 wide so the Pool z2 chain doesn't become a
    # serial ~6us drain after the final copies.
    Z2_LAST = _CFG.get(
        "z2_last", {1: "act", 4: "act", 7: "act", 2: "dve", 6: "dve"})
    AL = mybir.AluOpType

    with TileContext(nc) as tc:
        with tc.tile_pool(name="const", bufs=1) as cpool, \
             tc.tile_pool(name="xin", bufs=2) as xpool, \
             tc.tile_pool(name="work", bufs=8) as wpool, \
             tc.tile_pool(name="oout", bufs=2) as opool, \
             tc.tile_pool(name="psum", bufs=1, space="PSUM") as psum:
            rt = cpool.tile([128, N_CK], f32)
            wt = cpool.tile([128, N_CK], f32)
            nc.sync.dma_start(out=rt[:], in_=r_d.ap())
            nc.sync.dma_start(out=wt[:], in_=w_d.ap())
            # Per-(cl,m) matrix tiles: allocated here, loaded later (after
            # iteration 0's x slices) so the first theta blends aren't stuck
            # behind the whole 2.1MB constant load on the serial DMA bus.
            mt = {}
            for cl in range(C_LOC):
                for m in range(Or):
                    mt[cl, m] = cpool.tile([128, 2, 2, 128], f16,
                                           name=f"mt{cl}_{m}")

            def load_mats(cl, half):
                lo = half * (Or // 2)
                for m in range(lo, lo + Or // 2):
                    nc.sync.dma_start(out=mt[cl, m][:], in_=m_d.ap()[:, cl, m])

            # All 8 PSUM banks as one tile ([128,2,W] f32 = one 2KB bank per
            # group): the matmuls write the exact W window, PE can run up to
            # 8 groups ahead of the copies, and adjacent banks let one ACT
            # op copy two groups at once.
            U_all = psum.tile([128, 8, 2, W], f32, name="U_all")
            U_tiles = [U_all[:, i] for i in range(8)]
            uidx = 0
            # Persistent V ring (pairs of m): pads zeroed once (Pool
            # memsets); the ACT copies then only write the W-wide middles.
            V_tiles = []
            for i in range(8):
                V = cpool.tile([128, 2, 2, nu], f16, name=f"V{i}")
                for dm in range(2):
                    for u in range(2):
                        nc.gpsimd.memset(V[:, dm, u, 0:padl], 0.0)
                        nc.gpsimd.memset(V[:, dm, u, padl + W:nu], 0.0)
                V_tiles.append(V)
            vidx = 0

            iters = [(b, cl) for b in range(B) for cl in range(C_LOC)]

            def load_x(b, cl):
                x_sb = xpool.tile([128, Or, 2, W], f16, tag="x_sb",
                                  name="x_sb")
                src = x_d.ap()[b, cl].rearrange("k (u p) j -> p k u j", p=128)
                nc.sync.dma_start(out=x_sb[:], in_=src)
                return x_sb

            def emit_s1_m(cl, xs, m):
                """Theta blend for one (b,cl,m); returns the t tile."""
                cki = cl * Or + m
                z = wpool.tile([128, 2, W], f16, tag="z", name="z",
                               bufs=4)
                if m in Z1_ACT:
                    nc.scalar.mul(z[:], xs[:, (m + 1) % Or],
                                  rt[:, cki:cki + 1])
                elif m in Z1_POOL:
                    nc.gpsimd.tensor_scalar_mul(
                        z[:], xs[:, (m + 1) % Or], rt[:, cki:cki + 1])
                else:
                    nc.vector.tensor_scalar_mul(
                        z[:], xs[:, (m + 1) % Or], rt[:, cki:cki + 1])
                # Two iterations' worth of t tiles live at once (the
                # software pipeline emits S1(i+1) before tail(i)).
                t = wpool.tile([128, 2, W], f16, tag="t", name="t",
                               bufs=16)
                nc.vector.tensor_tensor(
                    out=t[:], in0=xs[:, m], in1=z[:], op=AL.add)
                return t

            def emit_s1(cl, xs):
                return [emit_s1_m(cl, xs, m) for m in range(Or)]

            def make_out_tiles():
                # Two half-size output tiles so the second store DMA can
                # start while m=4..7 are still being computed.
                out_a = opool.tile([128, Or // 2, 2, nv], f16, tag="out_a",
                                   name="out_a")
                out_b = opool.tile([128, Or // 2, 2, nv], f16, tag="out_b",
                                   name="out_b")
                return out_a, out_b

            def emit_tail_pair(b, cl, ts, m0, out_a, out_b, last):
                nonlocal uidx, vidx
                V = V_tiles[vidx % 8]
                vidx += 1
                g0 = uidx % 8
                for dm in range(2):
                    m = m0 + dm
                    U = U_tiles[uidx % 8]
                    uidx += 1
                    for u in range(2):
                        for v in range(2):
                            nc.tensor.matmul(
                                U[:, u],
                                mt[cl, m][:, u, v],
                                ts[m][:, v],
                                start=(v == 0), stop=(v == 1))
                # One ACT op stages both groups' W-wide rows PSUM->SBUF
                # (f32->f16, adjacent PSUM banks): 1040ns vs 2x613.
                nc.scalar.copy(V[:, :, :, padl:padl + W],
                               U_all[:, g0:g0 + 2])
                for dm in range(2):
                    m = m0 + dm
                    cki = cl * Or + m
                    z2 = wpool.tile([128, 2, nv], f16, tag="z2", name="z2",
                                    bufs=16)
                    eng = "pool"
                    if last:
                        eng = Z2_LAST.get(m, "pool")
                    elif m in Z2_ACT:
                        eng = "act"
                    if eng == "act":
                        nc.scalar.mul(z2[:], V[:, dm, :, 1:nu],
                                      wt[:, cki:cki + 1])
                    elif eng == "dve":
                        nc.vector.tensor_scalar_mul(
                            z2[:], V[:, dm, :, 1:nu], wt[:, cki:cki + 1])
                    else:
                        nc.gpsimd.tensor_scalar_mul(
                            z2[:], V[:, dm, :, 1:nu], wt[:, cki:cki + 1])
                    out_sb = out_a if m < Or // 2 else out_b
                    nc.vector.tensor_tensor(
                        out=out_sb[:, m % (Or // 2)], in0=V[:, dm, :, 0:nv],
                        in1=z2[:], op=AL.add)
                    q = 2 if (last and _CFG.get("out4_last", True)) else 4
                    if (m + 1) % q == 0:
                        lo = m + 1 - q
                        dst = o_d.ap()[b, cl, lo:lo + q].rearrange(
                            "k (u p) j -> p k u j", p=128)
                        nc.sync.dma_start(
                            out=dst,
                            in_=out_sb[:, lo % (Or // 2):lo % (Or // 2) + q])

            def emit_tail(b, cl, ts, last=False):
                out_a, out_b = make_out_tiles()
                for m0 in range(0, Or, 2):
                    emit_tail_pair(b, cl, ts, m0, out_a, out_b, last)

            def emit_s1_interleaved(cl_n, xs_n, b, cl, ts, last):
                """Per-pair interleave of S1(i+1) with tail(i)."""
                out_a, out_b = make_out_tiles()
                ts_n = []
                for m0 in range(0, Or, 2):
                    ts_n.append(emit_s1_m(cl_n, xs_n, m0))
                    ts_n.append(emit_s1_m(cl_n, xs_n, m0 + 1))
                    emit_tail_pair(b, cl, ts, m0, out_a, out_b, last)
                return ts_n

            # Software pipeline: emit iteration i+1's loads + theta blends
            # before iteration i's matmul/copy/x-blend block, so each
            # engine's in-order queue always has ready work at block edges.
            # Iteration 0 is fine-grained: half the x slices, then the first
            # mats chunk, then the rest — so the pipe fills ~2us earlier.
            if _CFG.get("fine_start", True):
                xs = xpool.tile([128, Or, 2, W], f16, tag="x_sb", name="x_sb")
                b0, c0 = iters[0]
                for half in range(2):
                    lo = half * (Or // 2)
                    src = x_d.ap()[b0, c0, lo:lo + Or // 2].rearrange(
                        "k (u p) j -> p k u j", p=128)
                    nc.sync.dma_start(out=xs[:, lo:lo + Or // 2], in_=src)
                    load_mats(0, half)
                ts = emit_s1(iters[0][1], xs)
            else:
                xs = load_x(*iters[0])
                load_mats(0, 0)
                load_mats(0, 1)
                ts = emit_s1(iters[0][1], xs)
            interleave = _CFG.get("interleave", False)
            for i in range(len(iters)):
                last = i == len(iters) - 1
                if i + 1 < len(iters):
                    xs_n = load_x(*iters[i + 1])
                    if i == 0:
                        load_mats(1, 0)
                        load_mats(1, 1)
                    if interleave:
                        ts_n = emit_s1_interleaved(
                            iters[i + 1][1], xs_n,
                            iters[i][0], iters[i][1], ts, last)
                    else:
                        ts_n = emit_s1(iters[i + 1][1], xs_n)
                        emit_tail(iters[i][0], iters[i][1], ts, last=last)
                else:
                    ts_n = None
                    emit_tail(iters[i][0], iters[i][1], ts, last=last)
                ts = ts_n
    nc.compile()
    return nc


_CFG = {}                         # build-config knobs (sim search)
_NC_CACHE = {}


def kernel(x, g0):
    x = np.ascontiguousarray(np.asarray(x, dtype=np.float32))
    g0 = np.asarray(g0, dtype=np.float32)
    tabs = _reference_tables(g0)
    padl, padr = _pads(tabs)
    nu = W + padl + padr

    if (padl, padr) not in _NC_CACHE:
        _NC_CACHE[(padl, padr)] = _build_program(padl, padr)
    nc = _NC_CACHE[(padl, padr)]

    in_maps = []
    slot_maps = []
    for core in range(N_CORES):
        channels = list(range(core * C_LOC, (core + 1) * C_LOC))
        mats, rscal, wxr, hvals, slot_to_k = _core_tables(tabs, channels, padl, nu)
        in_maps.append({
            "xs": np.ascontiguousarray(
                x[:, channels[0]:channels[-1] + 1]).astype(np.float16),
            "mats": mats.astype(np.float16),
            "rscal": rscal, "wx": wxr,
        })
        assert float(rscal.max()) < 3e4 and float(wxr.max()) < 3e4
        slot_maps.append((slot_to_k, hvals))

    res = bass_utils.run_bass_kernel_spmd(
        nc, in_maps, core_ids=list(range(N_CORES)),
        trace=bool(int(os.environ.get("KERNEL_TRACE", "0"))))
    kernel.last_results = res

    out = np.empty((B, C, Or, H, W), dtype=np.float32)
    for core in range(N_CORES):
        # [B, C_LOC, Or, H, nv] f16, slot m
        raw = res.results[core]["o"].astype(np.float32)
        slot_to_k, hvals = slot_maps[core]
        for cl in range(C_LOC):
            c = core * C_LOC + cl
            for m in range(Or):
                k = int(slot_to_k[cl, m])
                s = padl + int(hvals[cl, m])
                out[:, c, k] = raw[:, cl, m, :, s:s + W]
    return out

